# revision 42
# baseline (speedup 1.0000x reference)
import sys

sys.path.insert(0, "/opt/trn_rl_repo")

import numpy as np

import concourse.bass as bass
import concourse.bacc as bacc
import concourse.tile as tile
from concourse import mybir
from concourse.bass_utils import run_bass_kernel_spmd

B, S, H = 4096, 2048, 18
N_CORES = 8
BL = B // N_CORES  # 512 batch per core
N_D = 4
GAMMA = 0.5
NG = 2  # interleaved batch groups (pipelined chains)
NBLK = 6  # batch blocks packed into partitions per group
FD = 44  # free dim per block (2*6*44 = 528 >= 512)
GBL = NBLK * FD  # batch per group
HB = NBLK * H  # 108 hidden rows
NXR = NBLK + 1  # 6 x rows + 1 ones row
NBUF = 4
F32 = mybir.dt.float32
F32R = mybir.dt.float32r

_cache = {}


def _set_geom(ng, fd, nblk=6):
    global NG, FD, NBLK, GBL, HB, NXR
    NG, FD, NBLK = ng, fd, nblk
    GBL = NBLK * FD
    HB = NBLK * H
    NXR = NBLK + 1
    assert NG * GBL >= BL, (NG, GBL, BL)


# blob layout (columns): whh, wxb (on rows 0:NXR), fcw, per-group z0 blocks
def _blob_cols():
    c_whh = 0
    c_wxb = c_whh + HB
    c_fcw = c_wxb + HB
    c_z0 = c_fcw + NBLK
    return c_whh, c_wxb, c_fcw, c_z0, c_z0 + NG * FD


KCHUNK = 512  # max steps per launch (xbuf must fit in SBUF)


def _build(K, clamp_engines=("vector", "vector"), emit_h=False):
    nc = bacc.Bacc(None, target_bir_lowering=False, debug=True)
    c_whh, c_wxb, c_fcw, c_z0, CW = _blob_cols()
    blob = nc.declare_dram_parameter("blob", [HB, CW], F32R, isOutput=False)
    ublb = nc.declare_dram_parameter("ublb", [HB, 2], F32, isOutput=False)
    xbufs = [
        nc.declare_dram_parameter(f"xbuf{g}", [NXR, K * FD], F32R, isOutput=False)
        for g in range(NG)
    ]
    out = nc.declare_dram_parameter("out", [NBLK, NG * FD], F32, isOutput=True)
    if emit_h:
        outh = nc.declare_dram_parameter("outh", [HB, NG * FD], F32, isOutput=True)

    with tile.TileContext(nc) as tc:
        with (
            tc.tile_pool(name="singles", bufs=1) as singles,
            tc.tile_pool(
                name="psum", bufs=max(2, 8 // NG), space="PSUM"
            ) as psum_pool,
        ):
            blob_sb = singles.tile([HB, CW], F32R)
            ublb_sb = singles.tile([HB, 2], F32)
            xbuf_sb = [
                singles.tile([NXR, K * FD], F32R, name=f"xb{g}") for g in range(NG)
            ]
            # blob first (weights gate every step), split across queues
            nc.default_dma_engine.dma_start(out=blob_sb[:], in_=blob[:])
            nc.default_dma_engine.dma_start(out=xbuf_sb[0][:], in_=xbufs[0][:])
            if NG > 1:
                for g in range(1, NG):
                    nc.gpsimd.dma_start(out=xbuf_sb[g][:], in_=xbufs[g][:])
            nc.gpsimd.dma_start(out=ublb_sb[:], in_=ublb[:])

            whh_ap = blob_sb[0:HB, c_whh : c_whh + HB]
            wxb_ap = blob_sb[0:NXR, c_wxb : c_wxb + HB]
            fcw_ap = blob_sb[0:HB, c_fcw : c_fcw + NBLK]
            z0_aps = [
                blob_sb[0:HB, c_z0 + g * FD : c_z0 + (g + 1) * FD] for g in range(NG)
            ]
            ub_ap = ublb_sb[0:HB, 0:1]
            lb_ap = ublb_sb[0:HB, 1:2]

            states = [
                [singles.tile([HB, FD], F32R, name=f"g{g}st{i}") for i in range(NBUF)]
                for g in range(NG)
            ]
            def clamp(g, nxt):
                eng = getattr(nc, clamp_engines[g % len(clamp_engines)])
                eng.tensor_scalar(
                    out=nxt[:],
                    in0=nxt[:],
                    scalar1=ub_ap,
                    scalar2=lb_ap,
                    op0=mybir.AluOpType.min,
                    op1=mybir.AluOpType.max,
                )

            # step 0: h1 = clamp(tanh(z0)) straight from precomputed z0 in the
            # blob — no xbuf or state dependency, so compute starts the moment
            # the blob lands. Group order on the scalar queue staggers the
            # chains into anti-phase.
            for g in range(NG):
                nxt = states[g][1 % NBUF]
                nc.scalar.activation(
                    out=nxt[:],
                    in_=z0_aps[g],
                    func=mybir.ActivationFunctionType.Tanh,
                    scale=1.0,
                )
                clamp(g, nxt)

            def step(g, t):
                cur = states[g][t % NBUF]
                nxt = states[g][(t + 1) % NBUF]
                psum = psum_pool.tile([HB, FD], F32, name=f"ps{g}")
                # x/bias part first: no state dependency, so it runs ahead on
                # the in-order PE queue during the previous tanh/clamp
                nc.tensor.matmul(
                    psum[:],
                    lhsT=wxb_ap,
                    rhs=xbuf_sb[g][:, t * FD : (t + 1) * FD],
                    start=True,
                    stop=False,
                )
                nc.tensor.matmul(
                    psum[:], lhsT=whh_ap, rhs=cur[:], start=False, stop=True
                )
                nc.scalar.activation(
                    out=nxt[:],
                    in_=psum[:],
                    func=mybir.ActivationFunctionType.Tanh,
                    scale=1.0,
                )
                clamp(g, nxt)

            for t in range(1, K):
                for g in range(NG):
                    step(g, t)

            out_sb = singles.tile([NBLK, NG * FD], F32)
            for g in range(NG):
                final = states[g][K % NBUF]
                # reuse a loop psum slot (same name/shape) for the fc matmul
                psum_fc = psum_pool.tile([HB, FD], F32, name=f"ps{g}")
                nc.tensor.matmul(
                    psum_fc[0:NBLK, :],
                    lhsT=fcw_ap,
                    rhs=final[:],
                    start=True,
                    stop=True,
                )
                nc.scalar.activation(
                    out=out_sb[0:NBLK, g * FD : (g + 1) * FD],
                    in_=psum_fc[0:NBLK, :],
                    func=mybir.ActivationFunctionType.Copy,
                    scale=1.0,
                )
            nc.default_dma_engine.dma_start(out=out[:], in_=out_sb[:])
            if emit_h:
                outh_sb = singles.tile([HB, NG * FD], F32)
                for g in range(NG):
                    nc.vector.tensor_copy(
                        outh_sb[0:HB, g * FD : (g + 1) * FD],
                        states[g][K % NBUF][:],
                    )
                nc.default_dma_engine.dma_start(out=outh[:], in_=outh_sb[:])
    nc.compile()
    return nc


def _step_np(h, xt, W_ih, W_hh, b):
    z = np.outer(xt, W_ih) + h @ W_hh + b
    hn = np.tanh(z)
    return np.concatenate([hn[:, :N_D], np.clip(hn[:, N_D:], -GAMMA, GAMMA)], axis=1)


def _pick_K(x, W_ih, W_hh, b):
    # The recurrence is contractive when sigma_max(W_hh) < 1 (tanh and clip
    # are 1-Lipschitz), so the final state only depends on the last K inputs.
    # Probe the actual decay on the real input tail: propagate the extreme
    # corner states and h=0 and find where they merge.
    W_hh64 = np.asarray(W_hh, np.float64)
    rho = float(np.linalg.svd(W_hh64, compute_uv=False)[0])
    if rho >= 0.995:
        return S
    x = np.asarray(x, np.float32)
    W_ih_v = np.asarray(W_ih, np.float32).reshape(H)
    b_v = np.asarray(b, np.float32).reshape(H)
    W_hh32 = np.asarray(W_hh, np.float32)
    hmax = np.concatenate([np.ones(N_D), np.full(H - N_D, GAMMA)]).astype(np.float32)
    PROBE = min(S, 256)
    h_a = np.zeros((B, H), np.float32)
    h_b = np.tile(hmax, (B, 1))
    h_c = -h_b.copy()
    t0 = S - PROBE
    k_star = None
    for k in range(PROBE):
        xt = x[:, t0 + k]
        h_a = _step_np(h_a, xt, W_ih_v, W_hh32, b_v)
        h_b = _step_np(h_b, xt, W_ih_v, W_hh32, b_v)
        h_c = _step_np(h_c, xt, W_ih_v, W_hh32, b_v)
        d = max(np.abs(h_a - h_b).max(), np.abs(h_a - h_c).max())
        if d < 1e-6:
            k_star = k + 1
            break
    if k_star is None:
        # fall back to the rigorous worst-case bound
        C = float(np.sqrt((H - N_D) * GAMMA * GAMMA + N_D))
        return int(min(S, max(16, np.ceil(np.log(1e-6 / C) / np.log(rho) * 1.25))))
    return int(min(S, max(24, k_star + 6)))


def _make_inmaps(x, W_ih, W_hh, b, fc_w, K, t_start=None, h0=None):
    x = np.asarray(x, np.float32)
    if t_start is None:
        t_start = S - K
    perm = np.r_[N_D:H, 0:N_D]  # clamped units first within each block
    W_hh_p = np.asarray(W_hh, np.float32)[perm][:, perm]
    W_ih_p = np.asarray(W_ih, np.float32).reshape(H)[perm]
    b_p = np.asarray(b, np.float32).reshape(H)[perm]
    fc_w_p = np.asarray(fc_w, np.float32).reshape(H)[perm]

    c_whh, c_wxb, c_fcw, c_z0, CW = _blob_cols()
    blob = np.zeros((HB, CW), np.float32)
    for j in range(NBLK):
        r = slice(18 * j, 18 * j + 18)
        blob[r, c_whh + 18 * j : c_whh + 18 * j + 18] = W_hh_p
        blob[j, c_wxb + 18 * j : c_wxb + 18 * j + 18] = W_ih_p
        blob[NBLK, c_wxb + 18 * j : c_wxb + 18 * j + 18] = b_p
        blob[r, c_fcw + j] = fc_w_p

    n_c = H - N_D
    ublb = np.empty((HB, 2), np.float32)
    for j in range(NBLK):
        ublb[18 * j : 18 * j + n_c, 0] = GAMMA
        ublb[18 * j + n_c : 18 * j + 18, 0] = 2.0
    ublb[:, 1] = -ublb[:, 0]

    in_maps = []
    for c in range(N_CORES):
        xc = x[c * BL : (c + 1) * BL, t_start : t_start + K]  # [512, K]
        xp = np.zeros((NG * GBL, K), np.float32)
        xp[:BL] = xc
        # per-core blob: z0 = outer(W_ih, x_0) + b (+ h0 @ W_hh when chaining)
        cblob = blob.copy()
        x0 = xp[:, 0].reshape(NG, NBLK, FD)
        if h0 is not None:
            h0p = np.zeros((NG * GBL, H), np.float32)
            h0p[:BL] = np.asarray(h0, np.float32)[c * BL : (c + 1) * BL][:, perm]
            zh = (h0p @ W_hh_p).T.reshape(H, NG, NBLK, FD)
        for g in range(NG):
            for j in range(NBLK):
                z0 = np.outer(W_ih_p, x0[g, j]) + b_p[:, None]
                if h0 is not None:
                    z0 = z0 + zh[:, g, j]
                cblob[
                    18 * j : 18 * j + 18,
                    c_z0 + g * FD : c_z0 + (g + 1) * FD,
                ] = z0
        im = {"blob": cblob, "ublb": ublb}
        for g in range(NG):
            xg = xp[g * GBL : (g + 1) * GBL]  # [GBL, K]
            xbuf = np.empty((NXR, K * FD), np.float32)
            xbuf[:NBLK] = (
                xg.reshape(NBLK, FD, K).transpose(0, 2, 1).reshape(NBLK, K * FD)
            )
            xbuf[NBLK] = 1.0
            im[f"xbuf{g}"] = xbuf
        in_maps.append(im)
    return in_maps


def _gather_out(res):
    rows = []
    for c in range(N_CORES):
        arr = res[c]["out"].reshape(NBLK, NG, FD)  # [block, group, col]
        rows.append(np.transpose(arr, (1, 0, 2)).reshape(NG * GBL)[:BL])
    return np.concatenate(rows, axis=0)  # [B]


def _gather_h(res):
    rows = []
    for c in range(N_CORES):
        arr = res[c]["outh"].reshape(NBLK, H, NG, FD)  # [block, unit, group, col]
        rows.append(np.transpose(arr, (2, 0, 3, 1)).reshape(NG * GBL, H)[:BL])
    return np.concatenate(rows, axis=0)  # [B, H] (permuted units)


def _get_nc(Kc, emit_h):
    key = ("nc", Kc, emit_h)
    if key not in _cache:
        _cache[key] = _build(Kc, emit_h=emit_h)
    return _cache[key]


def kernel(x, W_ih, W_hh, b, fc_w, fc_b):
    K = _pick_K(x, W_ih, W_hh, b)
    _cache["K"] = K
    cores = list(range(N_CORES))
    perm = np.r_[N_D:H, 0:N_D]
    inv_perm = np.argsort(perm)

    if K <= KCHUNK:
        nc = _get_nc(K, False)
        in_maps = _make_inmaps(x, W_ih, W_hh, b, fc_w, K)
        res = run_bass_kernel_spmd(nc, in_maps, cores).results
        out = _gather_out(res)
    else:
        # xbuf for all K steps would overflow SBUF: chain <=KCHUNK-step
        # launches, passing the hidden state through the next chunk's z0
        nch = int(np.ceil(K / KCHUNK))
        sizes = [K // nch + (1 if i < K % nch else 0) for i in range(nch)]
        t0 = S - K
        h0 = None
        for Kc in sizes:
            nc = _get_nc(Kc, True)
            in_maps = _make_inmaps(x, W_ih, W_hh, b, fc_w, Kc, t0, h0)
            res = run_bass_kernel_spmd(nc, in_maps, cores).results
            h0 = _gather_h(res)[:, inv_perm]
            t0 += Kc
        out = _gather_out(res)

    return (out.reshape(B, 1) + np.asarray(fc_b, dtype=np.float32)).astype(
        np.float32
    )


# revision 56
# speedup vs baseline: 1.1583x; 1.1583x over previous
import sys

sys.path.insert(0, "/opt/trn_rl_repo")

import numpy as np

import concourse.bass as bass
import concourse.bacc as bacc
import concourse.tile as tile
from concourse import mybir
from concourse.bass_utils import run_bass_kernel_spmd

B, S, H = 4096, 2048, 18
N_CORES = 8
BL = B // N_CORES  # 512 batch per core
N_D = 4
GAMMA = 0.5
NG = 2  # interleaved batch groups (pipelined chains)
NBLK = 6  # batch blocks packed into partitions per group
FD = 44  # free dim per block (2*6*44 = 528 >= 512)
GBL = NBLK * FD  # batch per group
HB = NBLK * H  # 108 hidden rows
NXR = NBLK + 1  # 6 x rows + 1 ones row
NBUF = 4
F32 = mybir.dt.float32
F32R = mybir.dt.float32r

_cache = {}


def _set_geom(ng, fd, nblk=6):
    global NG, FD, NBLK, GBL, HB, NXR
    NG, FD, NBLK = ng, fd, nblk
    GBL = NBLK * FD
    HB = NBLK * H
    NXR = NBLK + 1
    assert NG * GBL >= BL, (NG, GBL, BL)


# blob layout (columns): whh, wxb (rows 0:NXR), fcw, ub, lb, per-group z0
def _blob_cols():
    c_whh = 0
    c_wxb = c_whh + HB
    c_fcw = c_wxb + HB
    c_ub = c_fcw + NBLK
    c_z0 = c_ub + 2
    return c_whh, c_wxb, c_fcw, c_ub, c_z0, c_z0 + NG * FD


KCHUNK = 512  # max steps per launch (xbuf must fit in SBUF)


def _build(K, clamp_engines=("vector", "vector"), emit_h=False):
    nc = bacc.Bacc(None, target_bir_lowering=False, debug=True)
    c_whh, c_wxb, c_fcw, c_ub, c_z0, CW = _blob_cols()
    blob = nc.declare_dram_parameter("blob", [HB, CW], F32R, isOutput=False)
    xbufs = [
        nc.declare_dram_parameter(f"xbuf{g}", [NXR, K * FD], F32R, isOutput=False)
        for g in range(NG)
    ]
    out = nc.declare_dram_parameter("out", [NBLK, NG * FD], F32, isOutput=True)
    if emit_h:
        outh = nc.declare_dram_parameter("outh", [HB, NG * FD], F32, isOutput=True)

    with tile.TileContext(nc) as tc:
        with (
            tc.tile_pool(name="singles", bufs=1) as singles,
            tc.tile_pool(
                name="psum", bufs=max(2, 8 // NG), space="PSUM"
            ) as psum_pool,
        ):
            blob_sb = singles.tile([HB, CW], F32R)
            xbuf_sb = [
                singles.tile([NXR, K * FD], F32R, name=f"xb{g}") for g in range(NG)
            ]
            # blob (weights+bounds+z0) and group-0 x on the fast HWDGE queue;
            # the other group's x alone on the gpsimd queue so neither queue
            # has anything critical waiting behind a large transfer
            nc.default_dma_engine.dma_start(out=blob_sb[:], in_=blob[:])
            nc.default_dma_engine.dma_start(out=xbuf_sb[0][:], in_=xbufs[0][:])
            for g in range(1, NG):
                nc.gpsimd.dma_start(out=xbuf_sb[g][:], in_=xbufs[g][:])

            whh_ap = blob_sb[0:HB, c_whh : c_whh + HB]
            wxb_ap = blob_sb[0:NXR, c_wxb : c_wxb + HB]
            fcw_ap = blob_sb[0:HB, c_fcw : c_fcw + NBLK]
            z0_aps = [
                blob_sb[0:HB, c_z0 + g * FD : c_z0 + (g + 1) * FD] for g in range(NG)
            ]
            ub_ap = blob_sb[0:HB, c_ub : c_ub + 1].bitcast(F32)
            lb_ap = blob_sb[0:HB, c_ub + 1 : c_ub + 2].bitcast(F32)

            states = [
                [singles.tile([HB, FD], F32R, name=f"g{g}st{i}") for i in range(NBUF)]
                for g in range(NG)
            ]
            def clamp(g, nxt):
                eng = getattr(nc, clamp_engines[g % len(clamp_engines)])
                eng.tensor_scalar(
                    out=nxt[:],
                    in0=nxt[:],
                    scalar1=ub_ap,
                    scalar2=lb_ap,
                    op0=mybir.AluOpType.min,
                    op1=mybir.AluOpType.max,
                )

            # step 0: h1 = clamp(tanh(z0)) straight from precomputed z0 in the
            # blob — no xbuf or state dependency, so compute starts the moment
            # the blob lands. Group order on the scalar queue staggers the
            # chains into anti-phase.
            for g in range(NG):
                nxt = states[g][1 % NBUF]
                nc.scalar.activation(
                    out=nxt[:],
                    in_=z0_aps[g],
                    func=mybir.ActivationFunctionType.Tanh,
                    scale=1.0,
                )
                clamp(g, nxt)

            def step(g, t):
                cur = states[g][t % NBUF]
                nxt = states[g][(t + 1) % NBUF]
                psum = psum_pool.tile([HB, FD], F32, name=f"ps{g}")
                # x/bias part first: no state dependency, so it runs ahead on
                # the in-order PE queue during the previous tanh/clamp
                nc.tensor.matmul(
                    psum[:],
                    lhsT=wxb_ap,
                    rhs=xbuf_sb[g][:, t * FD : (t + 1) * FD],
                    start=True,
                    stop=False,
                )
                nc.tensor.matmul(
                    psum[:], lhsT=whh_ap, rhs=cur[:], start=False, stop=True
                )
                nc.scalar.activation(
                    out=nxt[:],
                    in_=psum[:],
                    func=mybir.ActivationFunctionType.Tanh,
                    scale=1.0,
                )
                clamp(g, nxt)

            for t in range(1, K):
                for g in range(NG):
                    step(g, t)

            out_sb = singles.tile([NBLK, NG * FD], F32)
            for g in range(NG):
                final = states[g][K % NBUF]
                # reuse a loop psum slot (same name/shape) for the fc matmul
                psum_fc = psum_pool.tile([HB, FD], F32, name=f"ps{g}")
                nc.tensor.matmul(
                    psum_fc[0:NBLK, :],
                    lhsT=fcw_ap,
                    rhs=final[:],
                    start=True,
                    stop=True,
                )
                # split the PSUM evacuations across engines so they overlap
                if g % 2 == 0:
                    nc.scalar.activation(
                        out=out_sb[0:NBLK, g * FD : (g + 1) * FD],
                        in_=psum_fc[0:NBLK, :],
                        func=mybir.ActivationFunctionType.Copy,
                        scale=1.0,
                    )
                else:
                    nc.vector.tensor_copy(
                        out_sb[0:NBLK, g * FD : (g + 1) * FD], psum_fc[0:NBLK, :]
                    )
            nc.default_dma_engine.dma_start(out=out[:], in_=out_sb[:])
            if emit_h:
                outh_sb = singles.tile([HB, NG * FD], F32)
                for g in range(NG):
                    nc.vector.tensor_copy(
                        outh_sb[0:HB, g * FD : (g + 1) * FD],
                        states[g][K % NBUF][:],
                    )
                nc.default_dma_engine.dma_start(out=outh[:], in_=outh_sb[:])
    nc.compile()
    return nc


def _step_np(h, xt, W_ih, W_hh, b):
    z = np.outer(xt, W_ih) + h @ W_hh + b
    hn = np.tanh(z)
    return np.concatenate([hn[:, :N_D], np.clip(hn[:, N_D:], -GAMMA, GAMMA)], axis=1)


def _pick_K(x, W_ih, W_hh, b):
    # The recurrence is contractive when sigma_max(W_hh) < 1 (tanh and clip
    # are 1-Lipschitz), so the final state only depends on the last K inputs.
    # Probe the actual decay on the real input tail: propagate the extreme
    # corner states and h=0 and find where they merge.
    W_hh64 = np.asarray(W_hh, np.float64)
    rho = float(np.linalg.svd(W_hh64, compute_uv=False)[0])
    if rho >= 0.995:
        return S
    x = np.asarray(x, np.float32)
    W_ih_v = np.asarray(W_ih, np.float32).reshape(H)
    b_v = np.asarray(b, np.float32).reshape(H)
    W_hh32 = np.asarray(W_hh, np.float32)
    hmax = np.concatenate([np.ones(N_D), np.full(H - N_D, GAMMA)]).astype(np.float32)
    PROBE = min(S, 256)
    h_a = np.zeros((B, H), np.float32)
    h_b = np.tile(hmax, (B, 1))
    h_c = -h_b.copy()
    t0 = S - PROBE
    k_star = None
    for k in range(PROBE):
        xt = x[:, t0 + k]
        h_a = _step_np(h_a, xt, W_ih_v, W_hh32, b_v)
        h_b = _step_np(h_b, xt, W_ih_v, W_hh32, b_v)
        h_c = _step_np(h_c, xt, W_ih_v, W_hh32, b_v)
        d = max(np.abs(h_a - h_b).max(), np.abs(h_a - h_c).max())
        if d < 1e-6:
            k_star = k + 1
            break
    if k_star is None:
        # fall back to the rigorous worst-case bound
        C = float(np.sqrt((H - N_D) * GAMMA * GAMMA + N_D))
        return int(min(S, max(16, np.ceil(np.log(1e-6 / C) / np.log(rho) * 1.25))))
    return int(min(S, max(16, k_star + 2)))


def _make_inmaps(x, W_ih, W_hh, b, fc_w, K, t_start=None, h0=None):
    x = np.asarray(x, np.float32)
    if t_start is None:
        t_start = S - K
    perm = np.r_[N_D:H, 0:N_D]  # clamped units first within each block
    W_hh_p = np.asarray(W_hh, np.float32)[perm][:, perm]
    W_ih_p = np.asarray(W_ih, np.float32).reshape(H)[perm]
    b_p = np.asarray(b, np.float32).reshape(H)[perm]
    fc_w_p = np.asarray(fc_w, np.float32).reshape(H)[perm]

    c_whh, c_wxb, c_fcw, c_ub, c_z0, CW = _blob_cols()
    blob = np.zeros((HB, CW), np.float32)
    n_c = H - N_D
    for j in range(NBLK):
        r = slice(18 * j, 18 * j + 18)
        blob[r, c_whh + 18 * j : c_whh + 18 * j + 18] = W_hh_p
        blob[j, c_wxb + 18 * j : c_wxb + 18 * j + 18] = W_ih_p
        blob[NBLK, c_wxb + 18 * j : c_wxb + 18 * j + 18] = b_p
        blob[r, c_fcw + j] = fc_w_p
        blob[18 * j : 18 * j + n_c, c_ub] = GAMMA
        blob[18 * j + n_c : 18 * j + 18, c_ub] = 2.0
    blob[:, c_ub + 1] = -blob[:, c_ub]

    in_maps = []
    for c in range(N_CORES):
        xc = x[c * BL : (c + 1) * BL, t_start : t_start + K]  # [512, K]
        xp = np.zeros((NG * GBL, K), np.float32)
        xp[:BL] = xc
        # per-core blob: z0 = outer(W_ih, x_0) + b (+ h0 @ W_hh when chaining)
        cblob = blob.copy()
        x0 = xp[:, 0].reshape(NG, NBLK, FD)
        if h0 is not None:
            h0p = np.zeros((NG * GBL, H), np.float32)
            h0p[:BL] = np.asarray(h0, np.float32)[c * BL : (c + 1) * BL][:, perm]
            zh = (h0p @ W_hh_p).T.reshape(H, NG, NBLK, FD)
        for g in range(NG):
            for j in range(NBLK):
                z0 = np.outer(W_ih_p, x0[g, j]) + b_p[:, None]
                if h0 is not None:
                    z0 = z0 + zh[:, g, j]
                cblob[
                    18 * j : 18 * j + 18,
                    c_z0 + g * FD : c_z0 + (g + 1) * FD,
                ] = z0
        im = {"blob": cblob}
        for g in range(NG):
            xg = xp[g * GBL : (g + 1) * GBL]  # [GBL, K]
            xbuf = np.empty((NXR, K * FD), np.float32)
            xbuf[:NBLK] = (
                xg.reshape(NBLK, FD, K).transpose(0, 2, 1).reshape(NBLK, K * FD)
            )
            xbuf[NBLK] = 1.0
            im[f"xbuf{g}"] = xbuf
        in_maps.append(im)
    return in_maps


def _gather_out(res):
    rows = []
    for c in range(N_CORES):
        arr = res[c]["out"].reshape(NBLK, NG, FD)  # [block, group, col]
        rows.append(np.transpose(arr, (1, 0, 2)).reshape(NG * GBL)[:BL])
    return np.concatenate(rows, axis=0)  # [B]


def _gather_h(res):
    rows = []
    for c in range(N_CORES):
        arr = res[c]["outh"].reshape(NBLK, H, NG, FD)  # [block, unit, group, col]
        rows.append(np.transpose(arr, (2, 0, 3, 1)).reshape(NG * GBL, H)[:BL])
    return np.concatenate(rows, axis=0)  # [B, H] (permuted units)


def _get_nc(Kc, emit_h):
    key = ("nc", Kc, emit_h)
    if key not in _cache:
        _cache[key] = _build(Kc, emit_h=emit_h)
    return _cache[key]


def kernel(x, W_ih, W_hh, b, fc_w, fc_b):
    K = _pick_K(x, W_ih, W_hh, b)
    _cache["K"] = K
    cores = list(range(N_CORES))
    perm = np.r_[N_D:H, 0:N_D]
    inv_perm = np.argsort(perm)

    if K <= KCHUNK:
        nc = _get_nc(K, False)
        in_maps = _make_inmaps(x, W_ih, W_hh, b, fc_w, K)
        res = run_bass_kernel_spmd(nc, in_maps, cores).results
        out = _gather_out(res)
    else:
        # xbuf for all K steps would overflow SBUF: chain <=KCHUNK-step
        # launches, passing the hidden state through the next chunk's z0
        nch = int(np.ceil(K / KCHUNK))
        sizes = [K // nch + (1 if i < K % nch else 0) for i in range(nch)]
        t0 = S - K
        h0 = None
        for Kc in sizes:
            nc = _get_nc(Kc, True)
            in_maps = _make_inmaps(x, W_ih, W_hh, b, fc_w, Kc, t0, h0)
            res = run_bass_kernel_spmd(nc, in_maps, cores).results
            h0 = _gather_h(res)[:, inv_perm]
            t0 += Kc
        out = _gather_out(res)

    return (out.reshape(B, 1) + np.asarray(fc_b, dtype=np.float32)).astype(
        np.float32
    )
